# revision 1
# baseline (speedup 1.0000x reference)
"""Trainium2 Bass kernel for masked multi-head attention scores (softmax(QK^T)).

Reference computation (B=2, S=2048, D=768, H=12, DK=64):
    q = (query @ Wq.T + bq)  -> [B,H,S,DK]
    k = (key   @ Wk.T + bk)  -> [B,H,S,DK]
    scores = q @ k.T / sqrt(DK)            [B,H,S,S]
    m = where(mask == -10000, 1e9, 0)      [B,S]
    scores = scores - m[:,None,:,None] - m[:,None,None,:]
    out = softmax(scores, axis=-1)

Sharding: 8 cores = 2 batches x 4 head-groups (3 heads each). Each core gets
its batch's query^T/key^T (pre-transposed on host - pure layout change), its
3 heads' weight slices, and computes softmax scores for those heads.

Device algorithm per core:
  - Project q^T, k^T in [dk, s] layout via PE matmuls (weights pre-transposed
    host-side; 1/sqrt(DK) folded into Wq/bq as an exact power-of-2 scale).
  - Augment contraction dim 64 -> 66: q_aug = [q/8, -m', 1], k_aug =
    [k, 1, -m'] so the QK^T matmul itself applies both mask penalties.
    The f32 absorption (sigma - 1e9 rounds to exactly -1e9) reproduces the
    reference's masked-row arithmetic bit-for-bit.
  - softmax without a max-reduction: the reference's row-max is exactly
    -m'_row (masked rows: -1e9; unmasked: shift-invariant, any shift works
    since scores are O(10)), so exp(x + m'_row) via one ACT pass with a
    per-partition bias, with accum_out producing the row sums for free.
  - DVE: reciprocal of sums + per-row scale; DMA result tiles out.
"""

import os
import sys

import numpy as np

if not os.path.isdir(os.path.join(os.path.dirname(__file__), "concourse")):
    for _p in ("/opt/trn_rl_repo",):
        if os.path.isdir(_p) and _p not in sys.path:
            sys.path.insert(0, _p)

B, S, D, H = 2, 2048, 768, 12
DK = D // H  # 64
HPC = 3  # heads per core
N_CORES = 8
KAUG = DK + 2  # 66: contraction dim with the two mask/ones rows
NQ = S // 128  # 16 query tiles per head
NKC = D // 128  # 6 contraction chunks for the projections
NN = S // 512  # 4 free-dim chunks of 512 (fp32 moving-operand max)

SENTINEL = np.float32(-10000.0)
BIG = np.float32(1.0e9)

_NC = None
LAST_RESULTS = None


def _build_program():
    import concourse.bacc as bacc
    import concourse.mybir as mybir
    import concourse.tile as tile

    f32 = mybir.dt.float32
    AF = mybir.ActivationFunctionType

    nc = bacc.Bacc(
        "TRN2", target_bir_lowering=False, debug=False, enable_asserts=False
    )

    xqT = nc.dram_tensor("xqT", [D, S], f32, kind="ExternalInput").ap()
    xkT = nc.dram_tensor("xkT", [D, S], f32, kind="ExternalInput").ap()
    wq3 = nc.dram_tensor("wq3", [D, HPC * DK], f32, kind="ExternalInput").ap()
    wk3 = nc.dram_tensor("wk3", [D, HPC * DK], f32, kind="ExternalInput").ap()
    bq3 = nc.dram_tensor("bq3", [DK, HPC], f32, kind="ExternalInput").ap()
    bk3 = nc.dram_tensor("bk3", [DK, HPC], f32, kind="ExternalInput").ap()
    # maskaux rows: [0] = -m', [1] = ones, [2] = -m'
    maskaux = nc.dram_tensor("maskaux", [3, S], f32, kind="ExternalInput").ap()
    # mrow[p, i] = m'[i*128 + p]: per-query-row exp bias
    mrow = nc.dram_tensor("mrow", [128, NQ], f32, kind="ExternalInput").ap()
    out = nc.dram_tensor("out", [HPC, S, S], f32, kind="ExternalOutput").ap()

    with tile.TileContext(nc) as tc:
        with (
            tc.tile_pool(name="const", bufs=1) as const,
            tc.tile_pool(name="xin", bufs=1) as xin,
            tc.tile_pool(name="aug", bufs=1) as aug,
            tc.tile_pool(name="psum", bufs=2, space="PSUM") as psum,
            tc.tile_pool(name="work", bufs=3) as work,
            tc.tile_pool(name="stat", bufs=4) as stat,
        ):
            wq_sb = const.tile([128, NKC, HPC * DK], f32, tag="wq", name="wq_sb")
            wk_sb = const.tile([128, NKC, HPC * DK], f32, tag="wk", name="wk_sb")
            bq_sb = const.tile([DK, HPC], f32, tag="bq", name="bq_sb")
            bk_sb = const.tile([DK, HPC], f32, tag="bk", name="bk_sb")
            mrow_sb = const.tile([128, NQ], f32, tag="mrow", name="mrow_sb")
            for c in range(NKC):
                nc.sync.dma_start(out=wq_sb[:, c, :], in_=wq3[c * 128 : (c + 1) * 128, :])
                nc.sync.dma_start(out=wk_sb[:, c, :], in_=wk3[c * 128 : (c + 1) * 128, :])
            nc.sync.dma_start(out=bq_sb, in_=bq3)
            nc.sync.dma_start(out=bk_sb, in_=bk3)
            nc.sync.dma_start(out=mrow_sb, in_=mrow)

            xq_sb = xin.tile([128, NKC, S], f32, tag="xq", name="xq_sb")
            xk_sb = xin.tile([128, NKC, S], f32, tag="xk", name="xk_sb")
            for c in range(NKC):
                nc.sync.dma_start(out=xq_sb[:, c, :], in_=xqT[c * 128 : (c + 1) * 128, :])
                nc.sync.dma_start(out=xk_sb[:, c, :], in_=xkT[c * 128 : (c + 1) * 128, :])

            # Projections: q_aug^T / k_aug^T tiles [66, S] per head.
            qaugs = []
            kaugs = []
            for j in range(HPC):
                qa = aug.tile([KAUG, S], f32, tag=f"qa{j}", name=f"qa{j}")
                ka = aug.tile([KAUG, S], f32, tag=f"ka{j}", name=f"ka{j}")
                nc.sync.dma_start(out=qa[DK : DK + 2, :], in_=maskaux[0:2, :])
                nc.sync.dma_start(out=ka[DK : DK + 2, :], in_=maskaux[1:3, :])
                for xa, wa, ba, dst in (
                    (xq_sb, wq_sb, bq_sb, qa),
                    (xk_sb, wk_sb, bk_sb, ka),
                ):
                    ps = psum.tile([DK, S], f32, tag="ps", name="ps_proj")
                    for c in range(NKC):
                        for n in range(NN):
                            nc.tensor.matmul(
                                ps[:, n * 512 : (n + 1) * 512],
                                lhsT=wa[:, c, j * DK : (j + 1) * DK],
                                rhs=xa[:, c, n * 512 : (n + 1) * 512],
                                start=(c == 0),
                                stop=(c == NKC - 1),
                            )
                    nc.scalar.activation(
                        out=dst[0:DK, :],
                        in_=ps,
                        func=AF.Identity,
                        bias=ba[:, j : j + 1],
                        scale=1.0,
                    )
                qaugs.append(qa)
                kaugs.append(ka)

            # Scores + softmax.
            for j in range(HPC):
                qa = qaugs[j]
                ka = kaugs[j]
                for qi in range(NQ):
                    ps = psum.tile([128, S], f32, tag="ps", name="ps_sc")
                    for n in range(NN):
                        nc.tensor.matmul(
                            ps[:, n * 512 : (n + 1) * 512],
                            lhsT=qa[:, qi * 128 : (qi + 1) * 128],
                            rhs=ka[:, n * 512 : (n + 1) * 512],
                            start=True,
                            stop=True,
                        )
                    ex = work.tile([128, S], f32, tag="ex", name="ex")
                    rs = stat.tile([128, 1], f32, tag="rs", name="rs")
                    nc.scalar.activation(
                        out=ex,
                        in_=ps,
                        func=AF.Exp,
                        bias=mrow_sb[:, qi : qi + 1],
                        scale=1.0,
                        accum_out=rs,
                    )
                    nc.vector.reciprocal(rs, rs)
                    nc.vector.tensor_scalar_mul(out=ex, in0=ex, scalar1=rs)
                    nc.sync.dma_start(
                        out=out[j, qi * 128 : (qi + 1) * 128, :], in_=ex
                    )

    nc.compile()
    return nc


def _get_program():
    global _NC
    if _NC is None:
        _NC = _build_program()
    return _NC


def kernel(query, key, mask, Wq, bq, Wk, bk):
    global LAST_RESULTS
    from concourse.bass_utils import run_bass_kernel_spmd

    query = np.asarray(query, dtype=np.float32)
    key = np.asarray(key, dtype=np.float32)
    mask = np.asarray(mask, dtype=np.float32)
    Wq = np.asarray(Wq, dtype=np.float32)
    bq = np.asarray(bq, dtype=np.float32)
    Wk = np.asarray(Wk, dtype=np.float32)
    bk = np.asarray(bk, dtype=np.float32)

    nc = _get_program()

    ones_row = np.ones(S, dtype=np.float32)
    in_maps = []
    for core in range(N_CORES):
        b = core // 4
        g = core % 4
        rows = slice(g * HPC * DK, (g + 1) * HPC * DK)
        mprime = np.where(mask[b] == SENTINEL, BIG, np.float32(0.0)).astype(
            np.float32
        )
        neg_m = -mprime
        in_maps.append(
            {
                "xqT": np.ascontiguousarray(query[b].T),
                "xkT": np.ascontiguousarray(key[b].T),
                "wq3": np.ascontiguousarray(Wq[rows].T) * np.float32(0.125),
                "wk3": np.ascontiguousarray(Wk[rows].T),
                "bq3": np.ascontiguousarray(bq[rows].reshape(HPC, DK).T)
                * np.float32(0.125),
                "bk3": np.ascontiguousarray(bk[rows].reshape(HPC, DK).T),
                "maskaux": np.ascontiguousarray(
                    np.stack([neg_m, ones_row, neg_m])
                ),
                "mrow": np.ascontiguousarray(mprime.reshape(NQ, 128).T),
            }
        )

    trace = os.environ.get("BASS_KERNEL_TRACE") == "1"
    res = run_bass_kernel_spmd(
        nc, in_maps, core_ids=list(range(N_CORES)), trace=trace
    )
    LAST_RESULTS = res
    outs = np.stack([res.results[c]["out"] for c in range(N_CORES)])
    return outs.reshape(B, H, S, S)


# revision 14
# speedup vs baseline: 1.0997x; 1.0997x over previous
"""Trainium2 Bass kernel for masked multi-head attention scores (softmax(QK^T)).

Reference computation (B=2, S=2048, D=768, H=12, DK=64):
    q = (query @ Wq.T + bq)  -> [B,H,S,DK]
    k = (key   @ Wk.T + bk)  -> [B,H,S,DK]
    scores = q @ k.T / sqrt(DK)            [B,H,S,S]
    m = where(mask == -10000, 1e9, 0)      [B,S]
    scores = scores - m[:,None,:,None] - m[:,None,None,:]
    out = softmax(scores, axis=-1)

Sharding: 8 cores = 2 batches x 4 head-groups (3 heads each). Each core gets
its batch's query^T/key^T (pre-transposed on host - pure layout change), its
3 heads' weight slices, and computes softmax scores for those heads.

Device algorithm per core:
  - Project q^T, k^T into [dk, s] layout via PE matmuls, two heads packed
    per matmul (M=128). 1/sqrt(DK) is folded into Wq/bq as an exact
    power-of-2 scale. Inputs stream in [768, 512] column blocks.
  - QK^T runs as 2 float32r matmul passes per tile instead of the hardware
    fp32 mode's 4 half-speed passes: split q = qh + eq, k = kh + ek with
    fp32r rounding (11-bit mantissa hi, exact residual lo; qh+eq == q
    exactly). Pass 1 (K=66): qh*kh plus two mask rows that fold BOTH mask
    penalties into the matmul: q_aug = [qh, -M, 1], k_aug = [kh, 1, -M]
    with M = 2^30 (fp32r-exact; any huge exactly-representable value
    reproduces the reference's +-1e9 saturation since exp flushes it to 0).
    Pass 2 (K=128): [eq; qh] x [kh; ek] = eq*kh + qh*ek. Dropped eq*ek term
    is ~2^-26 relative. The f32 absorption (sigma - 2^30 rounds to exactly
    -2^30) reproduces the reference's masked-row arithmetic exactly.
  - softmax without a max-reduction: the reference's row-max equals the row
    penalty (masked rows: -M; unmasked rows: any shift works since scores
    are O(10)), so one ACT pass computes exp(x + M_row) via a per-partition
    bias, with accum_out producing row sums for free.
  - DVE: reciprocal of sums + per-row scale; DMA result tiles out.
"""

import os
import sys

import numpy as np

if not os.path.isdir(os.path.join(os.path.dirname(__file__), "concourse")):
    for _p in ("/opt/trn_rl_repo",):
        if os.path.isdir(_p) and _p not in sys.path:
            sys.path.insert(0, _p)

B, S, D, H = 2, 2048, 768, 12
DK = D // H  # 64
HPC = 3  # heads per core
N_CORES = 8
NQ = S // 128  # 16 query tiles per head
NKC = D // 128  # 6 contraction chunks for the projections
NN = S // 512  # 4 free-dim chunks of 512

SENTINEL = np.float32(-10000.0)
BIG = np.float32(2.0**30)

_NC = None
LAST_RESULTS = None


def _build_program():
    import concourse.bacc as bacc
    import concourse.mybir as mybir
    import concourse.tile as tile

    f32 = mybir.dt.float32
    f32r = mybir.dt.float32r
    AF = mybir.ActivationFunctionType

    nc = bacc.Bacc(
        "TRN2", target_bir_lowering=False, debug=False, enable_asserts=False
    )

    xqT = nc.dram_tensor("xqT", [D, S], f32, kind="ExternalInput").ap()
    xkT = nc.dram_tensor("xkT", [D, S], f32, kind="ExternalInput").ap()
    wq3 = nc.dram_tensor("wq3", [D, HPC * DK], f32, kind="ExternalInput").ap()
    wk3 = nc.dram_tensor("wk3", [D, HPC * DK], f32, kind="ExternalInput").ap()
    bq3 = nc.dram_tensor("bq3", [DK, HPC], f32, kind="ExternalInput").ap()
    bk3 = nc.dram_tensor("bk3", [DK, HPC], f32, kind="ExternalInput").ap()
    # maskaux rows: [0] = -M', [1] = ones, [2] = ones, [3] = -M'
    # (M' = 2^30 where masked; rows 0-1 feed q_aug, rows 2-3 feed k_aug)
    maskaux = nc.dram_tensor("maskaux", [4, S], f32, kind="ExternalInput").ap()
    # mrow[p, i] = M'[i*128 + p]: per-query-row exp bias
    mrow = nc.dram_tensor("mrow", [128, NQ], f32, kind="ExternalInput").ap()
    out = nc.dram_tensor("out", [HPC, S, S], f32, kind="ExternalOutput").ap()

    with tile.TileContext(nc) as tc:
        with (
            tc.tile_pool(name="const", bufs=1) as const,
            tc.tile_pool(name="aug", bufs=1) as aug,
            tc.tile_pool(name="psum", bufs=2, space="PSUM") as psum,
            tc.tile_pool(name="work", bufs=3) as work,
            tc.tile_pool(name="stat", bufs=4) as stat,
        ):
            wq_sb = const.tile([128, NKC, HPC * DK], f32, tag="wq", name="wq_sb")
            wk_sb = const.tile([128, NKC, HPC * DK], f32, tag="wk", name="wk_sb")
            bq_sb = const.tile([DK, HPC], f32, tag="bq", name="bq_sb")
            bk_sb = const.tile([DK, HPC], f32, tag="bk", name="bk_sb")
            mrow_sb = const.tile([128, NQ], f32, tag="mrow", name="mrow_sb")
            stgq_sb = const.tile([2, S], f32, tag="stgq", name="stgq_sb")
            stgk_sb = const.tile([2, S], f32, tag="stgk", name="stgk_sb")
            for c in range(NKC):
                nc.sync.dma_start(out=wq_sb[:, c, :], in_=wq3[c * 128 : (c + 1) * 128, :])
                nc.sync.dma_start(out=wk_sb[:, c, :], in_=wk3[c * 128 : (c + 1) * 128, :])
            nc.sync.dma_start(out=bq_sb, in_=bq3)
            nc.sync.dma_start(out=bk_sb, in_=bk3)
            nc.sync.dma_start(out=mrow_sb, in_=mrow)
            nc.sync.dma_start(out=stgq_sb, in_=maskaux[0:2, :])
            nc.sync.dma_start(out=stgk_sb, in_=maskaux[2:4, :])

            # Per-head score operands (float32r):
            # q66 = [qh(64); -M'(1); ones(1)], qcat = [eq(64); qh(64)]
            # k66 = [kh(64); ones(1); -M'(1)], kcat = [kh(64); ek(64)]
            q66s = [aug.tile([66, S], f32r, tag=f"q66_{j}", name=f"q66_{j}") for j in range(HPC)]
            k66s = [aug.tile([66, S], f32r, tag=f"k66_{j}", name=f"k66_{j}") for j in range(HPC)]
            qcats = [aug.tile([128, S], f32r, tag=f"qc_{j}", name=f"qc_{j}") for j in range(HPC)]
            kcats = [aug.tile([128, S], f32r, tag=f"kc_{j}", name=f"kc_{j}") for j in range(HPC)]
            for j in range(HPC):
                nc.vector.tensor_copy(out=q66s[j][DK : DK + 2, :], in_=stgq_sb)
                nc.vector.tensor_copy(out=k66s[j][DK : DK + 2, :], in_=stgk_sb)

            # ---- Projections (masters in fp32, then f32r hi/lo splits) ----
            with (
                tc.tile_pool(name="xio", bufs=2) as xio,
                tc.tile_pool(name="mast", bufs=1) as mast,
            ):
                for side, (xt, wa, ba) in enumerate(
                    ((xqT, wq_sb, bq_sb), (xkT, wk_sb, bk_sb))
                ):
                    masters = [
                        mast.tile([DK, S], f32, tag=f"m{j}", name=f"m{side}_{j}")
                        for j in range(HPC)
                    ]
                    for n in range(NN):
                        ns = slice(n * 512, (n + 1) * 512)
                        xn = xio.tile([128, NKC, 512], f32, tag="xn", name="xn")
                        for c in range(NKC):
                            nc.sync.dma_start(
                                out=xn[:, c, :], in_=xt[c * 128 : (c + 1) * 128, ns]
                            )
                        for j in range(HPC):
                            pj = psum.tile([DK, 512], f32, tag="ps", name="pj")
                            for c in range(NKC):
                                nc.tensor.matmul(
                                    pj,
                                    lhsT=wa[:, c, j * DK : (j + 1) * DK],
                                    rhs=xn[:, c, :],
                                    start=(c == 0),
                                    stop=(c == NKC - 1),
                                )
                            nc.scalar.activation(
                                out=masters[j][:, ns], in_=pj, func=AF.Identity,
                                bias=ba[:, j : j + 1], scale=1.0,
                            )
                    # f32r hi/lo splits (q = qh + eq exactly; same for k).
                    # Compute engines are partition-hardwired, so the halves
                    # that live at base partition 64 are filled via
                    # SBUF->SBUF DMA from base-0 sources.
                    for j in range(HPC):
                        msrc = masters[j]
                        t66 = (q66s if side == 0 else k66s)[j]
                        tcat = (qcats if side == 0 else kcats)[j]
                        hi = t66[0:DK, :]
                        nc.vector.tensor_copy(out=hi, in_=msrc)
                        if side == 0:
                            # qcat = [eq(0-63); qh(64-127)]
                            nc.vector.tensor_sub(
                                out=tcat[0:DK, :], in0=msrc, in1=hi.bitcast(f32)
                            )
                            nc.sync.dma_start(out=tcat[DK:128, :], in_=hi)
                        else:
                            # kcat = [kh(0-63); ek(64-127)]
                            nc.vector.tensor_copy(out=tcat[0:DK, :], in_=msrc)
                            ek = mast.tile([DK, S], f32r, tag="ek", name=f"ek{j}")
                            nc.vector.tensor_sub(
                                out=ek, in0=msrc, in1=hi.bitcast(f32)
                            )
                            nc.sync.dma_start(out=tcat[DK:128, :], in_=ek)

            # ---- Scores + softmax ----
            for j in range(HPC):
                q66, k66, qcat, kcat = q66s[j], k66s[j], qcats[j], kcats[j]
                for qi in range(NQ):
                    qs = slice(qi * 128, (qi + 1) * 128)
                    ps = psum.tile([128, S], f32, tag="ps", name="ps_sc")
                    for n in range(NN):
                        ns = slice(n * 512, (n + 1) * 512)
                        nc.tensor.matmul(
                            ps[:, ns], lhsT=q66[:, qs], rhs=k66[:, ns],
                            start=True, stop=False,
                        )
                        nc.tensor.matmul(
                            ps[:, ns], lhsT=qcat[:, qs], rhs=kcat[:, ns],
                            start=False, stop=True,
                        )
                    ex = work.tile([128, S], f32, tag="ex", name="ex")
                    rs = stat.tile([128, 1], f32, tag="rs", name="rs")
                    nc.scalar.activation(
                        out=ex, in_=ps, func=AF.Exp,
                        bias=mrow_sb[:, qi : qi + 1], scale=1.0,
                        accum_out=rs,
                    )
                    nc.vector.reciprocal(rs, rs)
                    nc.vector.tensor_scalar_mul(out=ex, in0=ex, scalar1=rs)
                    nc.sync.dma_start(out=out[j, qs, :], in_=ex)

    nc.compile()
    return nc


def _get_program():
    global _NC
    if _NC is None:
        _NC = _build_program()
    return _NC


def kernel(query, key, mask, Wq, bq, Wk, bk):
    global LAST_RESULTS
    from concourse.bass_utils import run_bass_kernel_spmd

    query = np.asarray(query, dtype=np.float32)
    key = np.asarray(key, dtype=np.float32)
    mask = np.asarray(mask, dtype=np.float32)
    Wq = np.asarray(Wq, dtype=np.float32)
    bq = np.asarray(bq, dtype=np.float32)
    Wk = np.asarray(Wk, dtype=np.float32)
    bk = np.asarray(bk, dtype=np.float32)

    nc = _get_program()

    ones_row = np.ones(S, dtype=np.float32)
    zeros64 = np.zeros(DK, dtype=np.float32)
    in_maps = []
    for core in range(N_CORES):
        b = core // 4
        g = core % 4
        rows = slice(g * HPC * DK, (g + 1) * HPC * DK)
        mprime = np.where(mask[b] == SENTINEL, BIG, np.float32(0.0)).astype(
            np.float32
        )
        neg_m = -mprime
        in_maps.append(
            {
                "xqT": np.ascontiguousarray(query[b].T),
                "xkT": np.ascontiguousarray(key[b].T),
                "wq3": np.ascontiguousarray(Wq[rows].T) * np.float32(0.125),
                "wk3": np.ascontiguousarray(Wk[rows].T),
                "bq3": np.ascontiguousarray(bq[rows].reshape(HPC, DK).T)
                * np.float32(0.125),
                "bk3": np.ascontiguousarray(bk[rows].reshape(HPC, DK).T),
                "maskaux": np.ascontiguousarray(
                    np.stack([neg_m, ones_row, ones_row, neg_m])
                ),
                "mrow": np.ascontiguousarray(mprime.reshape(NQ, 128).T),
            }
        )

    trace = os.environ.get("BASS_KERNEL_TRACE") == "1"
    res = run_bass_kernel_spmd(
        nc, in_maps, core_ids=list(range(N_CORES)), trace=trace
    )
    LAST_RESULTS = res
    outs = np.stack([res.results[c]["out"] for c in range(N_CORES)])
    return outs.reshape(B, H, S, S)


# revision 18
# speedup vs baseline: 1.2517x; 1.1382x over previous
"""Trainium2 Bass kernel for masked multi-head attention scores (softmax(QK^T)).

Reference computation (B=2, S=2048, D=768, H=12, DK=64):
    q = (query @ Wq.T + bq)  -> [B,H,S,DK]
    k = (key   @ Wk.T + bk)  -> [B,H,S,DK]
    scores = q @ k.T / sqrt(DK)            [B,H,S,S]
    m = where(mask == -10000, 1e9, 0)      [B,S]
    scores = scores - m[:,None,:,None] - m[:,None,None,:]
    out = softmax(scores, axis=-1)

Sharding: 8 cores = 2 batches x 4 head-groups (3 heads each). Each core gets
its batch's query^T/key^T (pre-transposed on host - pure layout change), its
3 heads' weight slices, and computes softmax scores for those heads.

Device algorithm per core:
  - Projections into [dk, s] layout via PE matmuls; heads 0+1 are packed
    into one M=128 matmul (they share the same rhs); 1/sqrt(DK) is folded
    into Wq/bq as an exact power-of-2 scale. K side runs first (scores need
    all of k); the Q side streams per 512-column slice with the scores for
    the covered query tiles interleaved, so ACT softmax work overlaps PE
    projection work.
  - QK^T runs as 2 float32r matmul passes per tile instead of the hardware
    fp32 mode's 4 half-speed passes: split q = qh + eq, k = kh + ek with
    fp32r rounding (11-bit-mantissa hi; the residual is exact, qh+eq == q).
    Pass 1 (K=66): qh*kh plus two mask rows folding BOTH mask penalties
    into the matmul: q_aug = [qh, -M, 1], k_aug = [kh, 1, -M] with
    M = 2^30 (fp32r-exact; any huge exactly-representable value reproduces
    the reference's +-1e9 saturation since exp flushes it to 0).
    Pass 2 (K=128): [eq; qh] x [kh; ek] = eq*kh + qh*ek. The dropped eq*ek
    term is ~2^-26 relative. The f32 absorption (sigma - 2^30 rounds to
    exactly -2^30) reproduces the reference's masked-row arithmetic.
  - Compute engines are partition-hardwired, so operand halves living at
    base partition 64 (and head 1's packed outputs) move via SBUF->SBUF
    DMA, which can shift partitions freely.
  - softmax without a max-reduction: the reference's row-max equals the row
    penalty (masked rows: -M; unmasked rows: any shift works since scores
    are O(10)), so one ACT pass computes exp(x + M_row) with a
    per-partition bias, with accum_out producing row sums for free.
  - DVE: reciprocal of sums + per-row scale; DMA result tiles out.
"""

import os
import sys

import numpy as np

if not os.path.isdir(os.path.join(os.path.dirname(__file__), "concourse")):
    for _p in ("/opt/trn_rl_repo",):
        if os.path.isdir(_p) and _p not in sys.path:
            sys.path.insert(0, _p)

B, S, D, H = 2, 2048, 768, 12
DK = D // H  # 64
HPC = 3  # heads per core
N_CORES = 8
NQ = S // 128  # 16 query tiles per head
NKC = D // 128  # 6 contraction chunks for the projections
NN = S // 512  # 4 free-dim chunks of 512

SENTINEL = np.float32(-10000.0)
BIG = np.float32(2.0**30)

_NC = None
LAST_RESULTS = None


def _build_program():
    import concourse.bacc as bacc
    import concourse.mybir as mybir
    import concourse.tile as tile

    f32 = mybir.dt.float32
    f32r = mybir.dt.float32r
    AF = mybir.ActivationFunctionType

    nc = bacc.Bacc(
        "TRN2", target_bir_lowering=False, debug=False, enable_asserts=False
    )

    xqT = nc.dram_tensor("xqT", [D, S], f32, kind="ExternalInput").ap()
    xkT = nc.dram_tensor("xkT", [D, S], f32, kind="ExternalInput").ap()
    wq3 = nc.dram_tensor("wq3", [D, HPC * DK], f32, kind="ExternalInput").ap()
    wk3 = nc.dram_tensor("wk3", [D, HPC * DK], f32, kind="ExternalInput").ap()
    # packed biases: col 0 = [b_h0; b_h1] (128), col 1 = [b_h2; zeros]
    bqs = nc.dram_tensor("bqs", [128, 2], f32, kind="ExternalInput").ap()
    bks = nc.dram_tensor("bks", [128, 2], f32, kind="ExternalInput").ap()
    # maskaux rows: [0] = -M', [1] = ones, [2] = ones, [3] = -M'
    maskaux = nc.dram_tensor("maskaux", [4, S], f32, kind="ExternalInput").ap()
    # mrow[p, i] = M'[i*128 + p]: per-query-row exp bias
    mrow = nc.dram_tensor("mrow", [128, NQ], f32, kind="ExternalInput").ap()
    out = nc.dram_tensor("out", [HPC, S, S], f32, kind="ExternalOutput").ap()

    with tile.TileContext(nc) as tc:
        with (
            tc.tile_pool(name="const", bufs=1) as const,
            tc.tile_pool(name="aug", bufs=1) as aug,
            tc.tile_pool(name="psum", bufs=2, space="PSUM") as psum,
            tc.tile_pool(name="xio", bufs=2) as xio,
            tc.tile_pool(name="mast", bufs=1) as mast,
            tc.tile_pool(name="qm", bufs=2) as qmp,
            tc.tile_pool(name="tmp", bufs=1) as tmp,
            tc.tile_pool(name="work", bufs=3) as work,
            tc.tile_pool(name="stat", bufs=4) as stat,
        ):
            wk_sb = const.tile([128, NKC, HPC * DK], f32, tag="wk", name="wk_sb")
            bk_sb = const.tile([128, 2], f32, tag="bk", name="bk_sb")
            wq_sb = const.tile([128, NKC, HPC * DK], f32, tag="wq", name="wq_sb")
            bq_sb = const.tile([128, 2], f32, tag="bq", name="bq_sb")
            mrow_sb = const.tile([128, NQ], f32, tag="mrow", name="mrow_sb")
            stgq_sb = const.tile([2, S], f32, tag="stgq", name="stgq_sb")
            stgk_sb = const.tile([2, S], f32, tag="stgk", name="stgk_sb")
            for c in range(NKC):
                nc.sync.dma_start(out=wk_sb[:, c, :], in_=wk3[c * 128 : (c + 1) * 128, :])
            nc.sync.dma_start(out=bk_sb, in_=bks)
            for c in range(NKC):
                nc.sync.dma_start(out=wq_sb[:, c, :], in_=wq3[c * 128 : (c + 1) * 128, :])
            nc.sync.dma_start(out=bq_sb, in_=bqs)
            nc.sync.dma_start(out=mrow_sb, in_=mrow)
            nc.sync.dma_start(out=stgq_sb, in_=maskaux[0:2, :])
            nc.sync.dma_start(out=stgk_sb, in_=maskaux[2:4, :])

            # Per-head score operands (float32r):
            # q66 = [qh(64); -M'(1); ones(1)], qcat = [eq(64); qh(64)]
            # k66 = [kh(64); ones(1); -M'(1)], kcat = [kh(64); ek(64)]
            q66s = [aug.tile([66, S], f32r, tag=f"q66_{j}", name=f"q66_{j}") for j in range(HPC)]
            k66s = [aug.tile([66, S], f32r, tag=f"k66_{j}", name=f"k66_{j}") for j in range(HPC)]
            qcats = [aug.tile([128, S], f32r, tag=f"qc_{j}", name=f"qc_{j}") for j in range(HPC)]
            kcats = [aug.tile([128, S], f32r, tag=f"kc_{j}", name=f"kc_{j}") for j in range(HPC)]
            for j in range(HPC):
                nc.vector.tensor_copy(out=q66s[j][DK : DK + 2, :], in_=stgq_sb)
                nc.vector.tensor_copy(out=k66s[j][DK : DK + 2, :], in_=stgk_sb)

            # ---- K-side projections (packed h0+h1, then h2) ----
            km01 = mast.tile([128, S], f32, tag="km01", name="km01")
            km2 = mast.tile([DK, S], f32, tag="km2", name="km2")
            for n in range(NN):
                ns = slice(n * 512, (n + 1) * 512)
                xn = xio.tile([128, NKC, 512], f32, tag="xn", name="xn")
                for c in range(NKC):
                    nc.sync.dma_start(
                        out=xn[:, c, :], in_=xkT[c * 128 : (c + 1) * 128, ns]
                    )
                p01 = psum.tile([128, 512], f32, tag="ps", name="p01")
                for c in range(NKC):
                    nc.tensor.matmul(
                        p01, lhsT=wk_sb[:, c, 0:128], rhs=xn[:, c, :],
                        start=(c == 0), stop=(c == NKC - 1),
                    )
                p2 = psum.tile([DK, 512], f32, tag="ps", name="p2")
                for c in range(NKC):
                    nc.tensor.matmul(
                        p2, lhsT=wk_sb[:, c, 128:192], rhs=xn[:, c, :],
                        start=(c == 0), stop=(c == NKC - 1),
                    )
                nc.scalar.activation(
                    out=km01[:, ns], in_=p01, func=AF.Identity,
                    bias=bk_sb[:, 0:1], scale=1.0,
                )
                nc.scalar.activation(
                    out=km2[:, ns], in_=p2, func=AF.Identity,
                    bias=bk_sb[0:DK, 1:2], scale=1.0,
                )

            # ---- K-side f32r hi/lo splits ----
            # DVE output partition base may differ from the input base; only
            # two SBUF *inputs* must share a base. So tensor_tensor for head 1
            # keeps both inputs at base 64 (via khtmp) and writes wherever.
            # h0 (base 0) and h2 (base 0)
            for j, msrc in ((0, km01[0:DK, :]), (2, km2[:, :])):
                hi = k66s[j][0:DK, :]
                nc.vector.tensor_copy(out=hi, in_=msrc)
                nc.vector.tensor_copy(out=kcats[j][0:DK, :], in_=msrc)
                nc.vector.tensor_sub(
                    out=kcats[j][DK:128, :], in0=msrc, in1=hi.bitcast(f32)
                )
            # h1 (packed output lives at base 64)
            khtmp = tmp.tile([128, S], f32r, tag="khtmp", name="khtmp")
            nc.vector.tensor_copy(out=khtmp[DK:128, :], in_=km01[DK:128, :])
            nc.vector.tensor_copy(out=k66s[1][0:DK, :], in_=khtmp[DK:128, :])
            nc.vector.tensor_copy(out=kcats[1][0:DK, :], in_=khtmp[DK:128, :])
            nc.vector.tensor_sub(
                out=kcats[1][DK:128, :], in0=km01[DK:128, :],
                in1=khtmp[DK:128, :].bitcast(f32),
            )

            # ---- Q-side projections per 512-column slice + scores ----
            for n in range(NN):
                ns = slice(n * 512, (n + 1) * 512)
                xn = xio.tile([128, NKC, 512], f32, tag="xn", name="xn")
                for c in range(NKC):
                    nc.sync.dma_start(
                        out=xn[:, c, :], in_=xqT[c * 128 : (c + 1) * 128, ns]
                    )
                q01 = psum.tile([128, 512], f32, tag="ps", name="q01")
                for c in range(NKC):
                    nc.tensor.matmul(
                        q01, lhsT=wq_sb[:, c, 0:128], rhs=xn[:, c, :],
                        start=(c == 0), stop=(c == NKC - 1),
                    )
                q2 = psum.tile([DK, 512], f32, tag="ps", name="q2")
                for c in range(NKC):
                    nc.tensor.matmul(
                        q2, lhsT=wq_sb[:, c, 128:192], rhs=xn[:, c, :],
                        start=(c == 0), stop=(c == NKC - 1),
                    )
                qm01 = qmp.tile([128, 512], f32, tag="qm01", name="qm01")
                qm2 = qmp.tile([DK, 512], f32, tag="qm2", name="qm2")
                nc.scalar.activation(
                    out=qm01, in_=q01, func=AF.Identity,
                    bias=bq_sb[:, 0:1], scale=1.0,
                )
                nc.scalar.activation(
                    out=qm2, in_=q2, func=AF.Identity,
                    bias=bq_sb[0:DK, 1:2], scale=1.0,
                )
                # splits for this slice
                for j, msrc in ((0, qm01[0:DK, :]), (2, qm2[:, :])):
                    hi = q66s[j][0:DK, ns]
                    nc.vector.tensor_copy(out=hi, in_=msrc)
                    nc.vector.tensor_sub(
                        out=qcats[j][0:DK, ns], in0=msrc, in1=hi.bitcast(f32)
                    )
                    nc.vector.tensor_copy(out=qcats[j][DK:128, ns], in_=hi)
                # h1: round at base 64 straight into qcat's hi half; shift
                # copies handle the base-0 destinations
                nc.vector.tensor_copy(out=qcats[1][DK:128, ns], in_=qm01[DK:128, :])
                nc.vector.tensor_copy(
                    out=q66s[1][0:DK, ns], in_=qcats[1][DK:128, ns]
                )
                nc.vector.tensor_sub(
                    out=qcats[1][0:DK, ns], in0=qm01[DK:128, :],
                    in1=qcats[1][DK:128, ns].bitcast(f32),
                )

                # scores for the query tiles covered by this slice
                for j in range(HPC):
                    q66, k66, qcat, kcat = q66s[j], k66s[j], qcats[j], kcats[j]
                    for qi in range(4 * n, 4 * n + 4):
                        qs = slice(qi * 128, (qi + 1) * 128)
                        ps = psum.tile([128, S], f32, tag="ps", name="ps_sc")
                        for m in range(NN):
                            ms = slice(m * 512, (m + 1) * 512)
                            nc.tensor.matmul(
                                ps[:, ms], lhsT=q66[:, qs], rhs=k66[:, ms],
                                start=True, stop=False,
                            )
                            nc.tensor.matmul(
                                ps[:, ms], lhsT=qcat[:, qs], rhs=kcat[:, ms],
                                start=False, stop=True,
                            )
                        ex = work.tile([128, S], f32, tag="ex", name="ex")
                        rs = stat.tile([128, 1], f32, tag="rs", name="rs")
                        nc.scalar.activation(
                            out=ex, in_=ps, func=AF.Exp,
                            bias=mrow_sb[:, qi : qi + 1], scale=1.0,
                            accum_out=rs,
                        )
                        nc.vector.reciprocal(rs, rs)
                        nc.vector.tensor_scalar_mul(out=ex, in0=ex, scalar1=rs)
                        nc.sync.dma_start(out=out[j, qs, :], in_=ex)

    nc.compile()
    return nc


def _get_program():
    global _NC
    if _NC is None:
        _NC = _build_program()
    return _NC


def kernel(query, key, mask, Wq, bq, Wk, bk):
    global LAST_RESULTS
    from concourse.bass_utils import run_bass_kernel_spmd

    query = np.asarray(query, dtype=np.float32)
    key = np.asarray(key, dtype=np.float32)
    mask = np.asarray(mask, dtype=np.float32)
    Wq = np.asarray(Wq, dtype=np.float32)
    bq = np.asarray(bq, dtype=np.float32)
    Wk = np.asarray(Wk, dtype=np.float32)
    bk = np.asarray(bk, dtype=np.float32)

    nc = _get_program()

    ones_row = np.ones(S, dtype=np.float32)
    zeros64 = np.zeros(DK, dtype=np.float32)
    in_maps = []
    for core in range(N_CORES):
        b = core // 4
        g = core % 4
        rows = slice(g * HPC * DK, (g + 1) * HPC * DK)
        mprime = np.where(mask[b] == SENTINEL, BIG, np.float32(0.0)).astype(
            np.float32
        )
        neg_m = -mprime
        bq3 = bq[rows] * np.float32(0.125)  # [192]
        bk3 = bk[rows]
        in_maps.append(
            {
                "xqT": np.ascontiguousarray(query[b].T),
                "xkT": np.ascontiguousarray(key[b].T),
                "wq3": np.ascontiguousarray(Wq[rows].T) * np.float32(0.125),
                "wk3": np.ascontiguousarray(Wk[rows].T),
                "bqs": np.ascontiguousarray(
                    np.stack(
                        [bq3[0:128], np.concatenate([bq3[128:192], zeros64])],
                        axis=1,
                    )
                ),
                "bks": np.ascontiguousarray(
                    np.stack(
                        [bk3[0:128], np.concatenate([bk3[128:192], zeros64])],
                        axis=1,
                    )
                ),
                "maskaux": np.ascontiguousarray(
                    np.stack([neg_m, ones_row, ones_row, neg_m])
                ),
                "mrow": np.ascontiguousarray(mprime.reshape(NQ, 128).T),
            }
        )

    trace = os.environ.get("BASS_KERNEL_TRACE") == "1"
    res = run_bass_kernel_spmd(
        nc, in_maps, core_ids=list(range(N_CORES)), trace=trace
    )
    LAST_RESULTS = res
    outs = np.stack([res.results[c]["out"] for c in range(N_CORES)])
    return outs.reshape(B, H, S, S)
